# revision 7
# baseline (speedup 1.0000x reference)
"""Trainium2 Bass kernel for multi-head self-attention.

Problem: B=8, N=2048, C=384, H=6 heads, D=64.
  qkv = x @ qkv_w.T + qkv_b ; q,k,v split; q *= D**-0.5
  attn = softmax(q @ k.T, axis=-1); out = (attn @ v) @ proj_w.T + proj_b

Sharding: pure data-parallel, one batch element per NeuronCore (8 cores).
No collectives needed.

Per-core layout strategy (everything stays in SBUF):
  - Host pre-transposes x -> xT [C, N] and weights to [in, out] layout (bf16).
  - k-bias dropped (softmax shift-invariant); v-bias folded into proj bias
    on the host; q scale folded into Wq/bq on the host.
  - qkT [2C, N] computed on-chip with C on partitions (heads at 64-partition
    offsets), v [N, C] in natural layout.
  - scores computed TRANSPOSED: s^T[m, q] = k^T.T @ q^T so the softmax
    reduction (over keys m) is along partitions -> done by a ones-matmul.
    Row-packed pairs of heads (K=64 at partition offsets 0/64) recover the
    half-empty systolic array.
  - exp on ScalarE (PSUM f32 -> SBUF bf16), no max-subtraction (|s| <~ 4).
  - numerator^T[d, q] = v.T @ e and denominator (broadcast over 64
    partitions) = ones.T @ e, both col-packed 2 heads/matmul, accumulated
    over m in PSUM.
  - normalize with DVE reciprocal + multiply -> aT [C, N] bf16, which is
    exactly the lhs-stream layout proj needs; proj output written
    transposed [C, N] and un-transposed on the host.
"""

import sys

sys.path.insert(0, "/opt/trn_rl_repo")

import numpy as np
import ml_dtypes

import concourse.bass as bass
import concourse.tile as tile
from concourse import bacc, mybir
from concourse.bass_utils import run_bass_kernel_spmd

B, N, C = 8, 2048, 384
H, D = 6, 64
SCALE = D ** -0.5
BF16 = mybir.dt.bfloat16
F32 = mybir.dt.float32
P = 128

NCORES = 8
NQC = N // 512          # 4 q-chunks of 512
NMT = N // P            # 16 m-tiles
QH = 1024               # q-half width for the attention inner loop

_NC = None
LAST_RESULT = None      # BassKernelResults of the most recent run


def _build_nc():
    nc = bacc.Bacc(
        "TRN2",
        target_bir_lowering=False,
        debug=False,
        enable_asserts=False,
        num_devices=NCORES,
    )

    xT_e = nc.declare_dram_parameter("xT", [C, N], BF16, isOutput=False)
    wqk_e = nc.declare_dram_parameter("wqkT", [C, 2 * C], BF16, isOutput=False)
    wv_e = nc.declare_dram_parameter("wvT", [C, C], BF16, isOutput=False)
    pw_e = nc.declare_dram_parameter("pwT", [C, C], BF16, isOutput=False)
    bq_e = nc.declare_dram_parameter("bq", [C, 1], F32, isOutput=False)
    bp_e = nc.declare_dram_parameter("bp", [C, 1], F32, isOutput=False)
    out_e = nc.declare_dram_parameter("out", [C, N], F32, isOutput=True)

    Exp = mybir.ActivationFunctionType.Exp

    with tile.TileContext(nc) as tc:
        from contextlib import ExitStack

        with ExitStack() as ctx:
            wpool = ctx.enter_context(tc.tile_pool(name="weights", bufs=1))
            xpool = ctx.enter_context(tc.tile_pool(name="xT", bufs=1))
            qkpool = ctx.enter_context(tc.tile_pool(name="qkT", bufs=1))
            vpool = ctx.enter_context(tc.tile_pool(name="v", bufs=1))
            apool = ctx.enter_context(tc.tile_pool(name="aT", bufs=1))
            epool = ctx.enter_context(tc.tile_pool(name="e", bufs=2))
            rpool = ctx.enter_context(tc.tile_pool(name="r", bufs=2))
            opool = ctx.enter_context(tc.tile_pool(name="o", bufs=2))

            # ---- load constants / inputs ----
            xT = []
            for k in range(3):
                t = xpool.tile([P, N], BF16, tag=f"xT{k}", name=f"xT{k}")
                nc.sync.dma_start(out=t[:], in_=xT_e[P * k : P * (k + 1), :])
                xT.append(t)
            wqk = []
            for k in range(3):
                t = wpool.tile([P, 2 * C], BF16, tag=f"wqk{k}", name=f"wqk{k}")
                nc.sync.dma_start(out=t[:], in_=wqk_e[P * k : P * (k + 1), :])
                wqk.append(t)
            wv = []
            for k in range(3):
                t = wpool.tile([P, C], BF16, tag=f"wv{k}", name=f"wv{k}")
                nc.sync.dma_start(out=t[:], in_=wv_e[P * k : P * (k + 1), :])
                wv.append(t)
            pw = []
            for k in range(3):
                t = wpool.tile([P, C], BF16, tag=f"pw{k}", name=f"pw{k}")
                nc.sync.dma_start(out=t[:], in_=pw_e[P * k : P * (k + 1), :])
                pw.append(t)
            bq = []
            bp = []
            for j in range(3):
                t = wpool.tile([P, 1], F32, tag=f"bq{j}", name=f"bq{j}")
                nc.sync.dma_start(out=t[:], in_=bq_e[P * j : P * (j + 1), :])
                bq.append(t)
                t = wpool.tile([P, 1], F32, tag=f"bp{j}", name=f"bp{j}")
                nc.sync.dma_start(out=t[:], in_=bp_e[P * j : P * (j + 1), :])
                bp.append(t)

            # ---- phase 1: qkT[2C, N] = (wqkT).T @ xT, bias on q rows ----
            qkT = [qkpool.tile([P, N], BF16, tag=f"qkT{m}", name=f"qkT{m}") for m in range(6)]
            with tc.tile_pool(name="ps1", bufs=2, space="PSUM") as ps1:
                for mo in range(6):
                    ps = ps1.tile([P, N], F32, tag="qk_ps", name="qk_ps")
                    for qc in range(NQC):
                        for k in range(3):
                            nc.tensor.matmul(
                                ps[:, 512 * qc : 512 * (qc + 1)],
                                wqk[k][:, P * mo : P * (mo + 1)],
                                xT[k][:, 512 * qc : 512 * (qc + 1)],
                                start=(k == 0),
                                stop=(k == 2),
                            )
                    if mo < 3:
                        nc.vector.tensor_scalar_add(qkT[mo][:], ps[:], bq[mo][:])
                    else:
                        nc.vector.tensor_copy(qkT[mo][:], ps[:])

            # ---- phase 2: v_aug[N, 6*128] = per head [v_h | ones] (even) or
            # ---- [ones | v_h] (odd), so the nd-matmul puts the numerator on
            # ---- the partitions the proj layout needs and the denominator
            # ---- (64x replicated) on the other 64.
            vaug = [
                vpool.tile([P, H * P], BF16, tag=f"va{m}", name=f"va{m}")
                for m in range(NMT)
            ]
            with tc.tile_pool(name="ps2", bufs=4, space="PSUM") as ps2:
                for mt in range(NMT):
                    nc.vector.memset(vaug[mt][:], 1.0)
                    ps = ps2.tile([P, C], F32, tag="v_ps", name="v_ps")
                    for k in range(3):
                        nc.tensor.matmul(
                            ps[:],
                            xT[k][:, P * mt : P * (mt + 1)],
                            wv[k][:],
                            start=(k == 0),
                            stop=(k == 2),
                        )
                    for h in range(H):
                        off = P * h + (0 if h % 2 == 0 else D)
                        nc.vector.tensor_copy(
                            vaug[mt][:, off : off + D],
                            ps[:, D * h : D * (h + 1)],
                        )

            # ---- phase 3: attention, one head at a time, q in halves ----
            aT = [apool.tile([P, N], BF16, tag=f"aT{t}", name=f"aT{t}") for t in range(3)]
            with tc.tile_pool(name="ps3", bufs=2, space="PSUM") as ps3:
                for h in range(H):
                    qt = qkT[h // 2]
                    kt = qkT[3 + h // 2]
                    hp = slice(0, 64) if h % 2 == 0 else slice(64, 128)
                    # within nd: numerator partitions == hp, denominator = other
                    num_p = hp
                    den_p = slice(64, 128) if h % 2 == 0 else slice(0, 64)
                    for qh in range(2):
                        nd = ps3.tile([P, QH], F32, tag="nd", name="nd")

                        def emit_s_exp(mt):
                            ms = slice(P * mt, P * (mt + 1))
                            s = ps3.tile([P, QH], F32, tag="s", name="s")
                            for c in range(2):
                                qs = slice(QH * qh + 512 * c, QH * qh + 512 * (c + 1))
                                cs = slice(512 * c, 512 * (c + 1))
                                nc.tensor.matmul(
                                    s[:, cs], kt[hp, ms], qt[hp, qs],
                                    start=True, stop=True,
                                )
                            e = epool.tile([P, QH], BF16, tag="e", name="e")
                            nc.scalar.activation(e[:], s[:], Exp)
                            return e

                        def emit_nd(mt, e):
                            for c in range(2):
                                cs = slice(512 * c, 512 * (c + 1))
                                nc.tensor.matmul(
                                    nd[:, cs],
                                    vaug[mt][:, P * h : P * (h + 1)],
                                    e[:, cs],
                                    start=(mt == 0), stop=(mt == NMT - 1),
                                )

                        # 1-deep software pipeline: queue s(mt+1) on PE before
                        # nd(mt) so nd's wait on exp(mt) doesn't stall PE
                        e_prev = emit_s_exp(0)
                        for mt in range(1, NMT):
                            e_cur = emit_s_exp(mt)
                            emit_nd(mt - 1, e_prev)
                            e_prev = e_cur
                        emit_nd(NMT - 1, e_prev)

                        # normalize: r = 1/den on den partitions, DMA-shift the
                        # 64 replicated rows onto the numerator partitions,
                        # then aT[num rows] = num * r
                        r = rpool.tile([P, QH], F32, tag="r", name="r")
                        nc.vector.reciprocal(r[den_p, :], nd[den_p, :])
                        nc.sync.dma_start(out=r[num_p, :], in_=r[den_p, :])
                        nc.vector.tensor_mul(
                            aT[h // 2][num_p, QH * qh : QH * (qh + 1)],
                            nd[num_p, :],
                            r[num_p, :],
                        )

            # ---- phase 4: out^T[C, N] = pwT.T @ aT + bp ----
            with tc.tile_pool(name="ps4", bufs=2, space="PSUM") as ps4:
                for mo in range(3):
                    ps = ps4.tile([P, N], F32, tag="pj", name="pj")
                    for qc in range(NQC):
                        for k in range(3):
                            nc.tensor.matmul(
                                ps[:, 512 * qc : 512 * (qc + 1)],
                                pw[k][:, P * mo : P * (mo + 1)],
                                aT[k][:, 512 * qc : 512 * (qc + 1)],
                                start=(k == 0),
                                stop=(k == 2),
                            )
                    o = opool.tile([P, N], F32, tag="o", name="o")
                    nc.vector.tensor_scalar_add(o[:], ps[:], bp[mo][:])
                    nc.sync.dma_start(
                        out=out_e[P * mo : P * (mo + 1), :], in_=o[:]
                    )

    nc.compile()
    return nc


def _get_nc():
    global _NC
    if _NC is None:
        _NC = _build_nc()
    return _NC


def kernel(x, qkv_w, qkv_b, proj_w, proj_b, h=None, w=None, _trace=False):
    global LAST_RESULT
    x = np.asarray(x, dtype=np.float32)
    qkv_w = np.asarray(qkv_w, dtype=np.float32)
    qkv_b = np.asarray(qkv_b, dtype=np.float32)
    proj_w = np.asarray(proj_w, dtype=np.float32)
    proj_b = np.asarray(proj_b, dtype=np.float32)

    bf16 = ml_dtypes.bfloat16
    # q-scale folded into Wq/bq; k-bias dropped (softmax shift-invariant);
    # v-bias folded into the proj bias (attention rows sum to 1).
    wqkT = np.concatenate(
        [qkv_w[:C] * SCALE, qkv_w[C : 2 * C]], axis=0
    ).T.astype(bf16).copy()                        # [C, 2C]
    wvT = qkv_w[2 * C :].T.astype(bf16).copy()     # [C, C]
    pwT = proj_w.T.astype(bf16).copy()             # [C, C]
    bq = (qkv_b[:C] * SCALE).astype(np.float32).reshape(C, 1)
    bp = (proj_b + qkv_b[2 * C :] @ proj_w.T).astype(np.float32).reshape(C, 1)

    common = {"wqkT": wqkT, "wvT": wvT, "pwT": pwT, "bq": bq, "bp": bp}
    in_maps = []
    for i in range(NCORES):
        xT = np.ascontiguousarray(x[i].T).astype(bf16)
        in_maps.append({"xT": xT, **common})

    nc = _get_nc()
    res = run_bass_kernel_spmd(
        nc, in_maps, core_ids=list(range(NCORES)), trace=_trace
    )
    LAST_RESULT = res

    out = np.empty((B, N, C), dtype=np.float32)
    for i in range(NCORES):
        out[i] = res.results[i]["out"].T
    return out


if __name__ == "__main__":
    rng = np.random.default_rng(0)
    x = rng.standard_normal((B, N, C), dtype=np.float32)
    s = 1.0 / np.sqrt(C)
    qkv_w = rng.uniform(-s, s, (3 * C, C)).astype(np.float32)
    qkv_b = rng.uniform(-s, s, (3 * C,)).astype(np.float32)
    proj_w = rng.uniform(-s, s, (C, C)).astype(np.float32)
    proj_b = rng.uniform(-s, s, (C,)).astype(np.float32)
    out = kernel(x, qkv_w, qkv_b, proj_w, proj_b, 64, 32)
    print("out", out.shape, out.dtype, float(np.abs(out).mean()))


# revision 10
# speedup vs baseline: 1.4177x; 1.4177x over previous
"""Trainium2 Bass kernel for multi-head self-attention.

Problem: B=8, N=2048, C=384, H=6 heads, D=64.
  qkv = x @ qkv_w.T + qkv_b ; q,k,v split; q *= D**-0.5
  attn = softmax(q @ k.T, axis=-1); out = (attn @ v) @ proj_w.T + proj_b

Sharding: pure data-parallel, one batch element per NeuronCore (8 cores).
No collectives needed.

Per-core layout strategy (everything stays in SBUF):
  - Host pre-transposes x -> xT [C, N] and weights to [in, out] layout (bf16).
  - k-bias dropped (softmax shift-invariant); v-bias folded into proj bias
    on the host; q scale folded into Wq/bq on the host.
  - qkT [2C, N] computed on-chip with C on partitions (heads at 64-partition
    offsets), v [N, C] in natural layout.
  - scores computed TRANSPOSED: s^T[m, q] = k^T.T @ q^T so the softmax
    reduction (over keys m) is along partitions -> done by a ones-matmul.
    Row-packed pairs of heads (K=64 at partition offsets 0/64) recover the
    half-empty systolic array.
  - exp on ScalarE (PSUM f32 -> SBUF bf16), no max-subtraction (|s| <~ 4).
  - numerator^T[d, q] = v.T @ e and denominator (broadcast over 64
    partitions) = ones.T @ e, both col-packed 2 heads/matmul, accumulated
    over m in PSUM.
  - normalize with DVE reciprocal + multiply -> aT [C, N] bf16, which is
    exactly the lhs-stream layout proj needs; proj output written
    transposed [C, N] and un-transposed on the host.
"""

import sys

sys.path.insert(0, "/opt/trn_rl_repo")

import numpy as np
import ml_dtypes

import concourse.bass as bass
import concourse.tile as tile
from concourse import bacc, mybir
from concourse.bass_utils import run_bass_kernel_spmd

B, N, C = 8, 2048, 384
H, D = 6, 64
SCALE = D ** -0.5
BF16 = mybir.dt.bfloat16
F32 = mybir.dt.float32
P = 128

NCORES = 8
NQC = N // 512          # 4 q-chunks of 512
NMT = N // P            # 16 m-tiles
QH = 1024               # q-half width for the attention inner loop

_NC = None
LAST_RESULT = None      # BassKernelResults of the most recent run


def _build_nc():
    nc = bacc.Bacc(
        "TRN2",
        target_bir_lowering=False,
        debug=False,
        enable_asserts=False,
        num_devices=NCORES,
    )

    xT_e = nc.declare_dram_parameter("xT", [C, N], BF16, isOutput=False)
    wqk_e = nc.declare_dram_parameter("wqkT", [C, 2 * C], BF16, isOutput=False)
    wv_e = nc.declare_dram_parameter("wvT", [C, C], BF16, isOutput=False)
    pw_e = nc.declare_dram_parameter("pwT", [C, C], BF16, isOutput=False)
    bq_e = nc.declare_dram_parameter("bq", [C, 1], F32, isOutput=False)
    bp_e = nc.declare_dram_parameter("bp", [C, 1], F32, isOutput=False)
    out_e = nc.declare_dram_parameter("out", [C, N], F32, isOutput=True)

    Exp = mybir.ActivationFunctionType.Exp

    with tile.TileContext(nc) as tc:
        from contextlib import ExitStack

        with ExitStack() as ctx:
            wpool = ctx.enter_context(tc.tile_pool(name="weights", bufs=1))
            xpool = ctx.enter_context(tc.tile_pool(name="xT", bufs=1))
            qkpool = ctx.enter_context(tc.tile_pool(name="qkT", bufs=1))
            vpool = ctx.enter_context(tc.tile_pool(name="v", bufs=1))
            apool = ctx.enter_context(tc.tile_pool(name="aT", bufs=1))
            epool = ctx.enter_context(tc.tile_pool(name="e", bufs=2))
            rpool = ctx.enter_context(tc.tile_pool(name="r", bufs=2))
            opool = ctx.enter_context(tc.tile_pool(name="o", bufs=2))

            # ---- load constants / inputs ----
            xT = []
            for k in range(3):
                t = xpool.tile([P, N], BF16, tag=f"xT{k}", name=f"xT{k}")
                nc.sync.dma_start(out=t[:], in_=xT_e[P * k : P * (k + 1), :])
                xT.append(t)
            wqk = []
            for k in range(3):
                t = wpool.tile([P, 2 * C], BF16, tag=f"wqk{k}", name=f"wqk{k}")
                nc.sync.dma_start(out=t[:], in_=wqk_e[P * k : P * (k + 1), :])
                wqk.append(t)
            wv = []
            for k in range(3):
                t = wpool.tile([P, C], BF16, tag=f"wv{k}", name=f"wv{k}")
                nc.sync.dma_start(out=t[:], in_=wv_e[P * k : P * (k + 1), :])
                wv.append(t)
            pw = []
            for k in range(3):
                t = wpool.tile([P, C], BF16, tag=f"pw{k}", name=f"pw{k}")
                nc.sync.dma_start(out=t[:], in_=pw_e[P * k : P * (k + 1), :])
                pw.append(t)
            bq = []
            bp = []
            for j in range(3):
                t = wpool.tile([P, 1], F32, tag=f"bq{j}", name=f"bq{j}")
                nc.sync.dma_start(out=t[:], in_=bq_e[P * j : P * (j + 1), :])
                bq.append(t)
                t = wpool.tile([P, 1], F32, tag=f"bp{j}", name=f"bp{j}")
                nc.sync.dma_start(out=t[:], in_=bp_e[P * j : P * (j + 1), :])
                bp.append(t)

            # ---- phase 1: q^T/k^T with the head duplicated on both 64-row
            # ---- halves (K=128 scores matmuls keep the PE array fully active
            # ---- so the HAM clock-gate stays at 2.4 GHz; q is pre-halved on
            # ---- the host so the doubled contraction sums to the true score).
            qdup = [qkpool.tile([P, N], BF16, tag=f"qd{m}", name=f"qd{m}") for m in range(6)]
            kdup = [qkpool.tile([P, N], BF16, tag=f"kd{m}", name=f"kd{m}") for m in range(6)]
            with tc.tile_pool(name="ps1", bufs=2, space="PSUM") as ps1:
                for mo in range(6):
                    ps = ps1.tile([P, N], F32, tag="qk_ps", name="qk_ps")
                    for qc in range(NQC):
                        for k in range(3):
                            nc.tensor.matmul(
                                ps[:, 512 * qc : 512 * (qc + 1)],
                                wqk[k][:, P * mo : P * (mo + 1)],
                                xT[k][:, 512 * qc : 512 * (qc + 1)],
                                start=(k == 0),
                                stop=(k == 2),
                            )
                    if mo < 3:
                        dst = qdup
                        nc.vector.tensor_scalar_add(
                            dst[2 * mo][0:64, :], ps[0:64, :], bq[mo][0:64, :]
                        )
                        nc.vector.tensor_scalar_add(
                            dst[2 * mo + 1][64:128, :], ps[64:128, :], bq[mo][64:128, :]
                        )
                    else:
                        dst = kdup
                        mk = mo - 3
                        nc.vector.tensor_copy(dst[2 * mk][0:64, :], ps[0:64, :])
                        nc.vector.tensor_copy(dst[2 * mk + 1][64:128, :], ps[64:128, :])
                # duplicate each head onto the other 64-partition half via DMA
                for hh in range(6):
                    if hh % 2 == 0:
                        nc.sync.dma_start(out=qdup[hh][64:128, :], in_=qdup[hh][0:64, :])
                        nc.sync.dma_start(out=kdup[hh][64:128, :], in_=kdup[hh][0:64, :])
                    else:
                        nc.sync.dma_start(out=qdup[hh][0:64, :], in_=qdup[hh][64:128, :])
                        nc.sync.dma_start(out=kdup[hh][0:64, :], in_=kdup[hh][64:128, :])

            # ---- phase 2: v_aug[N, 6*128] = per head [v_h | ones] (even) or
            # ---- [ones | v_h] (odd), so the nd-matmul puts the numerator on
            # ---- the partitions the proj layout needs and the denominator
            # ---- (64x replicated) on the other 64.
            vaug = [
                vpool.tile([P, H * P], BF16, tag=f"va{m}", name=f"va{m}")
                for m in range(NMT)
            ]
            with tc.tile_pool(name="ps2", bufs=4, space="PSUM") as ps2:
                for mt in range(NMT):
                    nc.vector.memset(vaug[mt][:], 1.0)
                    ps = ps2.tile([P, C], F32, tag="v_ps", name="v_ps")
                    for k in range(3):
                        nc.tensor.matmul(
                            ps[:],
                            xT[k][:, P * mt : P * (mt + 1)],
                            wv[k][:],
                            start=(k == 0),
                            stop=(k == 2),
                        )
                    for h in range(H):
                        off = P * h + (0 if h % 2 == 0 else D)
                        nc.vector.tensor_copy(
                            vaug[mt][:, off : off + D],
                            ps[:, D * h : D * (h + 1)],
                        )

            # ---- phase 3: attention, one head at a time, q in halves ----
            aT = [apool.tile([P, N], BF16, tag=f"aT{t}", name=f"aT{t}") for t in range(3)]
            with tc.tile_pool(name="ps3", bufs=2, space="PSUM") as ps3:
                for h in range(H):
                    qt = qdup[h]
                    kt = kdup[h]
                    hp = slice(0, 128)
                    num_p = slice(0, 64) if h % 2 == 0 else slice(64, 128)
                    den_p = slice(64, 128) if h % 2 == 0 else slice(0, 64)
                    for qh in range(2):
                        nd = ps3.tile([P, QH], F32, tag="nd", name="nd")

                        def emit_s_exp(mt):
                            ms = slice(P * mt, P * (mt + 1))
                            s = ps3.tile([P, QH], F32, tag="s", name="s")
                            for c in range(2):
                                qs = slice(QH * qh + 512 * c, QH * qh + 512 * (c + 1))
                                cs = slice(512 * c, 512 * (c + 1))
                                nc.tensor.matmul(
                                    s[:, cs], kt[hp, ms], qt[hp, qs],
                                    start=True, stop=True,
                                )
                            e = epool.tile([P, QH], BF16, tag="e", name="e")
                            nc.scalar.activation(e[:], s[:], Exp)
                            return e

                        def emit_nd(mt, e):
                            for c in range(2):
                                cs = slice(512 * c, 512 * (c + 1))
                                nc.tensor.matmul(
                                    nd[:, cs],
                                    vaug[mt][:, P * h : P * (h + 1)],
                                    e[:, cs],
                                    start=(mt == 0), stop=(mt == NMT - 1),
                                )

                        # 1-deep software pipeline: queue s(mt+1) on PE before
                        # nd(mt) so nd's wait on exp(mt) doesn't stall PE
                        e_prev = emit_s_exp(0)
                        for mt in range(1, NMT):
                            e_cur = emit_s_exp(mt)
                            emit_nd(mt - 1, e_prev)
                            e_prev = e_cur
                        emit_nd(NMT - 1, e_prev)

                        # normalize: r = 1/den on den partitions, DMA-shift the
                        # 64 replicated rows onto the numerator partitions,
                        # then aT[num rows] = num * r
                        r = rpool.tile([P, QH], F32, tag="r", name="r")
                        nc.vector.reciprocal(r[den_p, :], nd[den_p, :])
                        nc.sync.dma_start(out=r[num_p, :], in_=r[den_p, :])
                        nc.vector.tensor_mul(
                            aT[h // 2][num_p, QH * qh : QH * (qh + 1)],
                            nd[num_p, :],
                            r[num_p, :],
                        )

            # ---- phase 4: out^T[C, N] = pwT.T @ aT + bp ----
            with tc.tile_pool(name="ps4", bufs=2, space="PSUM") as ps4:
                for mo in range(3):
                    ps = ps4.tile([P, N], F32, tag="pj", name="pj")
                    for qc in range(NQC):
                        for k in range(3):
                            nc.tensor.matmul(
                                ps[:, 512 * qc : 512 * (qc + 1)],
                                pw[k][:, P * mo : P * (mo + 1)],
                                aT[k][:, 512 * qc : 512 * (qc + 1)],
                                start=(k == 0),
                                stop=(k == 2),
                            )
                    o = opool.tile([P, N], F32, tag="o", name="o")
                    nc.vector.tensor_scalar_add(o[:], ps[:], bp[mo][:])
                    nc.sync.dma_start(
                        out=out_e[P * mo : P * (mo + 1), :], in_=o[:]
                    )

    nc.compile()
    return nc


def _get_nc():
    global _NC
    if _NC is None:
        _NC = _build_nc()
    return _NC


def kernel(x, qkv_w, qkv_b, proj_w, proj_b, h=None, w=None, _trace=False):
    global LAST_RESULT
    x = np.asarray(x, dtype=np.float32)
    qkv_w = np.asarray(qkv_w, dtype=np.float32)
    qkv_b = np.asarray(qkv_b, dtype=np.float32)
    proj_w = np.asarray(proj_w, dtype=np.float32)
    proj_b = np.asarray(proj_b, dtype=np.float32)

    bf16 = ml_dtypes.bfloat16
    # q-scale folded into Wq/bq; k-bias dropped (softmax shift-invariant);
    # v-bias folded into the proj bias (attention rows sum to 1).
    wqkT = np.concatenate(
        [qkv_w[:C] * (SCALE * 0.5), qkv_w[C : 2 * C]], axis=0
    ).T.astype(bf16).copy()                        # [C, 2C]
    wvT = qkv_w[2 * C :].T.astype(bf16).copy()     # [C, C]
    pwT = proj_w.T.astype(bf16).copy()             # [C, C]
    bq = (qkv_b[:C] * (SCALE * 0.5)).astype(np.float32).reshape(C, 1)
    bp = (proj_b + qkv_b[2 * C :] @ proj_w.T).astype(np.float32).reshape(C, 1)

    common = {"wqkT": wqkT, "wvT": wvT, "pwT": pwT, "bq": bq, "bp": bp}
    in_maps = []
    for i in range(NCORES):
        xT = np.ascontiguousarray(x[i].T).astype(bf16)
        in_maps.append({"xT": xT, **common})

    nc = _get_nc()
    import os as _os

    kw = {}
    if _os.environ.get("KEEP_TMPDIR"):
        kw["tmpdir"] = _os.environ["KEEP_TMPDIR"]
    res = run_bass_kernel_spmd(
        nc, in_maps, core_ids=list(range(NCORES)), trace=_trace, **kw
    )
    LAST_RESULT = res

    out = np.empty((B, N, C), dtype=np.float32)
    for i in range(NCORES):
        out[i] = res.results[i]["out"].T
    return out


if __name__ == "__main__":
    rng = np.random.default_rng(0)
    x = rng.standard_normal((B, N, C), dtype=np.float32)
    s = 1.0 / np.sqrt(C)
    qkv_w = rng.uniform(-s, s, (3 * C, C)).astype(np.float32)
    qkv_b = rng.uniform(-s, s, (3 * C,)).astype(np.float32)
    proj_w = rng.uniform(-s, s, (C, C)).astype(np.float32)
    proj_b = rng.uniform(-s, s, (C,)).astype(np.float32)
    out = kernel(x, qkv_w, qkv_b, proj_w, proj_b, 64, 32)
    print("out", out.shape, out.dtype, float(np.abs(out).mean()))
